# revision 1
# baseline (speedup 1.0000x reference)
"""BitNet MLP (ternary gate/up GEMM + silu*up + Hadamard + act-quant + down GEMM)
on 8 Trainium2 NeuronCores — v2.

Token-data-parallel (T=1024 tokens/core) like the baseline, restructured for
engine balance and overlap:
  - weights arrive HOST-TRANSPOSED (wgT/wuT [H,I], wdT [I,H]) so GEMM lhsT
    tiles load directly from HBM: no on-device weight transposes and no
    PSUM->SBUF weight copies,
  - ternarize = 3 elementwise passes split scalar/vector/gpsimd, straight to
    bf16 {-1,0,1},
  - H128 factor of the FWHT runs inside the GEMM1 epilogue (f32 matmul on the
    f32 silu*up product, before the single bf16 rounding) so the serial FWHT
    tail only contains the H64 butterflies + act-quant,
  - epilogue uses the Silu LUT directly (not sigmoid+mult),
  - FWHT token-slices are split DVE / gpsimd and GEMM2 starts after the first
    token half's quant completes (per-half scale broadcasts).
All quantization arithmetic is bit-identical to the baseline kernel.
"""

import sys

sys.path.insert(0, "/opt/trn_rl_repo")

import numpy as np

import concourse.bass as bass
import concourse.mybir as mybir
import concourse.tile as tile
from concourse import bacc, bass_isa

F32 = mybir.dt.float32
BF16 = mybir.dt.bfloat16
AX = mybir.AxisListType.X
OP = mybir.AluOpType
ACT_FN = mybir.ActivationFunctionType

MAGIC = 12582912.0  # 1.5*2^23: (x + MAGIC) - MAGIC == rint(x) in f32
EPS = 1e-5
QCLIP = 127.4375


def hadamard128():
    h = np.array([[1.0]], dtype=np.float32)
    while h.shape[0] < 128:
        h = np.block([[h, h], [h, -h]]).astype(np.float32)
    return h


def build_program(T, H, I, n_cores, sub=8):
    from concourse.masks import make_identity

    P = 128
    HC = H // P                    # 16 h-chunks
    C = I // P                     # 64 i-chunks (fwht H64 factor)
    NT = min(512, T)
    TTN = T // NT                  # 2 token tiles for matmul
    n_tb = T // P                  # 8 token 128-blocks
    n_sub = T // sub               # fwht token-slices
    L = int(np.log2(C))
    assert 2 ** L == C and T % P == 0 and H % 1024 == 0 and I % P == 0
    inv_sqrt_i = float(1.0 / np.sqrt(I))
    wcount = float(I) * float(H)
    HH = H // 1024                 # f32 1024-col stage chunks per ic-block: 2

    nc = bacc.Bacc("TRN2", target_bir_lowering=False, num_devices=n_cores)

    x_d = nc.dram_tensor("x_s", [T, H], F32, kind="ExternalInput")
    wgt_d = nc.dram_tensor("wgt", [H, I], F32, kind="ExternalInput")
    wut_d = nc.dram_tensor("wut", [H, I], F32, kind="ExternalInput")
    wdt_d = nc.dram_tensor("wdt", [I, H], F32, kind="ExternalInput")
    wgs_d = nc.dram_tensor("wg_s", [H // n_cores, I], F32, kind="ExternalInput")
    wus_d = nc.dram_tensor("wu_s", [H // n_cores, I], F32, kind="ExternalInput")
    wds_d = nc.dram_tensor("wd_s", [I // n_cores, H], F32, kind="ExternalInput")
    hm_d = nc.dram_tensor("hmat", [P, P], F32, kind="ExternalInput")
    out_d = nc.dram_tensor("out_s", [T, H], F32, kind="ExternalOutput")

    cc_in = nc.dram_tensor("cc_in", [1, 4], F32)
    cc_out = nc.dram_tensor("cc_out", [1, 4], F32, addr_space="Shared")

    # DRAM views for transposed-tile loads: [p, q, i] with h = q*128 + p
    wgt_v = wgt_d.ap().rearrange("(q p) i -> p q i", p=P)
    wut_v = wut_d.ap().rearrange("(q p) i -> p q i", p=P)
    wdt_v = wdt_d.ap().rearrange("(c p) h -> p c h", p=P)

    with tile.TileContext(nc) as tc:
        with (
            tc.tile_pool(name="consts", bufs=1) as consts,
            tc.tile_pool(name="wpipe", bufs=2) as wpipe,    # f32 stage [P,8,128]
            tc.tile_pool(name="wqp", bufs=1) as wqp,        # wq bf16 (g+u)
            tc.tile_pool(name="wdqp", bufs=1) as wdqp,      # wdq bf16 halves
            tc.tile_pool(name="xq", bufs=1) as xqp,         # xqT persistent
            tc.tile_pool(name="ip", bufs=1) as ip,          # interm
            tc.tile_pool(name="s2k", bufs=1) as s2k,        # f32 [P,512] scratch
            tc.tile_pool(name="sc", bufs=1) as sc,          # scales/rows
            tc.tile_pool(name="ot", bufs=1) as otp,         # out transpose tiles
            tc.tile_pool(name="ps_mm", bufs=4, space="PSUM") as ps_mm,
            tc.tile_pool(name="ps_h", bufs=2, space="PSUM") as ps_h,
            tc.tile_pool(name="ps_tp", bufs=2, space="PSUM") as ps_tp,
        ):
            # ---------------- constants
            hmat = consts.tile([P, P], F32, tag="hmat")
            nc.sync.dma_start(hmat[:], hm_d.ap())
            ident_f = consts.tile([P, P], F32, tag="ident_f")
            make_identity(nc, ident_f[:])
            ident_b = consts.tile([P, P], BF16, tag="ident_b")
            nc.vector.tensor_copy(ident_b[:], ident_f[:])
            magicB = consts.tile([P, 1], F32, tag="magicB")
            nc.vector.memset(magicB[:], MAGIC)
            nmagicB = consts.tile([P, 1], F32, tag="nmagicB")
            nc.vector.memset(nmagicB[:], -MAGIC)
            ones1 = consts.tile([1, P], F32, tag="ones1")
            nc.vector.memset(ones1[:], 1.0)
            halfn = consts.tile([P, 1], F32, tag="halfn")
            nc.vector.memset(halfn[:], -0.5)
            halfp = consts.tile([P, 1], F32, tag="halfp")
            nc.vector.memset(halfp[:], 0.5)

            # ---------------- weight-scale pass (shard |w| sums + AllReduce)
            def shard_abs_sum(src_d, rows, cols, tag):
                ntr, ntc = rows // P, cols // 1024
                acc = sc.tile([P, ntr * ntc], F32, tag=f"acc_{tag}")
                for r in range(ntr):
                    for q in range(ntc):
                        t = wpipe.tile([P, 8, P], F32, tag="wf32",
                                       name=f"as_{tag}_{r}_{q}")
                        nc.sync.dma_start(
                            t[:].rearrange("p a b -> p (a b)"),
                            src_d.ap()[r * P:(r + 1) * P,
                                       q * 1024:(q + 1) * 1024])
                        nc.vector.tensor_reduce(
                            out=acc[:, r * ntc + q:r * ntc + q + 1],
                            in_=t[:].rearrange("p a b -> p (a b)"),
                            op=OP.add, axis=AX, apply_absolute_value=True)
                tot = sc.tile([P, 1], F32, tag=f"tot_{tag}")
                nc.vector.tensor_reduce(out=tot[:], in_=acc[:], op=OP.add,
                                        axis=AX)
                red = sc.tile([P, 1], F32, tag=f"red_{tag}")
                nc.gpsimd.partition_all_reduce(
                    red[:], tot[:], channels=P, reduce_op=bass_isa.ReduceOp.add)
                return red

            red_g = shard_abs_sum(wgs_d, H // n_cores, I, "g")
            red_u = shard_abs_sum(wus_d, H // n_cores, I, "u")
            red_d = shard_abs_sum(wds_d, I // n_cores, H, "d")

            ccin_sb = sc.tile([1, 4], F32, tag="ccin")
            nc.vector.memset(ccin_sb[:], 0.0)
            nc.vector.tensor_copy(ccin_sb[:, 0:1], red_g[0:1, :])
            nc.vector.tensor_copy(ccin_sb[:, 1:2], red_u[0:1, :])
            nc.vector.tensor_copy(ccin_sb[:, 2:3], red_d[0:1, :])
            nc.sync.dma_start(cc_in.ap(), ccin_sb[:])
            nc.gpsimd.collective_compute(
                "AllReduce", OP.add, ins=[cc_in.ap()], outs=[cc_out.ap()],
                replica_groups=[list(range(n_cores))])
            sums_sb = sc.tile([1, 4], F32, tag="sums")
            nc.sync.dma_start(sums_sb[:], cc_out.ap())

            # ---------------- x: act-quant + transpose into xqT (bf16 ints)
            # (emitted before the scale-finalize chain so DVE/ACT/PE overlap
            # the AllReduce latency)
            xqt = xqp.tile([P, HC, T], BF16, tag="xqt")
            am_row = sc.tile([1, T], F32, tag="am_row")
            for tb in range(n_tb):
                xt = [wpipe.tile([P, 8, P], F32, tag="wf32", name=f"x{tb}_{q}")
                      for q in range(2)]
                for q in range(2):
                    nc.sync.dma_start(
                        xt[q][:].rearrange("p a b -> p (a b)"),
                        x_d.ap()[tb * P:(tb + 1) * P,
                                 q * 1024:(q + 1) * 1024])
                am2 = sc.tile([P, 2], F32, tag="am2")
                for q in range(2):
                    nc.vector.tensor_reduce(
                        out=am2[:, q:q + 1],
                        in_=xt[q][:].rearrange("p a b -> p (a b)"),
                        op=OP.max, axis=AX, apply_absolute_value=True)
                amc = sc.tile([P, 1], F32, tag="amc")
                nc.vector.tensor_reduce(out=amc[:], in_=am2[:], op=OP.max,
                                        axis=AX)
                nc.vector.tensor_scalar(amc[:], amc[:], EPS, None, OP.max)
                sx = sc.tile([P, 1], F32, tag="sx")
                nc.vector.reciprocal(sx[:], amc[:])
                nc.vector.tensor_scalar(sx[:], sx[:], 128.0, None, OP.mult)
                for q in range(2):
                    f = xt[q][:].rearrange("p a b -> p (a b)")
                    nc.scalar.activation(f, f, ACT_FN.Identity,
                                         bias=magicB[:], scale=sx[:])
                    xqb = s2k.tile([P, 1024], BF16, tag="ta",
                                   name=f"xqb{tb}_{q}")
                    nc.vector.tensor_scalar(xqb[:], f, -MAGIC, 127.0,
                                            OP.add, OP.min)
                    for k in range(8):
                        hc = q * 8 + k
                        pt = ps_tp.tile([P, P], BF16, tag="tp")
                        nc.tensor.transpose(pt[:], xqb[:, k * P:(k + 1) * P],
                                            ident_b[:])
                        nc.scalar.copy(xqt[:, hc, tb * P:(tb + 1) * P], pt[:])
                pr = ps_tp.tile([P, P], F32, tag="tp")
                nc.tensor.transpose(pr[:1, :], amc[:], ident_f[:])
                nc.scalar.copy(am_row[:, tb * P:(tb + 1) * P], pr[:1, :])

            # ---------------- finalize weight scales (waits on AllReduce)
            wm_row = sc.tile([1, 4], F32, tag="wm_row")
            nc.vector.tensor_scalar(wm_row[:], sums_sb[:], 1.0 / wcount, EPS,
                                    OP.mult, OP.max)
            ws_row = sc.tile([1, 4], F32, tag="ws_row")
            nc.vector.reciprocal(ws_row[:], wm_row[:])
            wsB = sc.tile([P, 4], F32, tag="wsB")
            nc.gpsimd.partition_broadcast(wsB[:], ws_row[:])

            # bcast = per-token gate dequant (am * wm_g / 128), all partitions
            sg = sc.tile([1, 1], F32, tag="sg")
            nc.vector.tensor_scalar(sg[:], wm_row[:, 0:1], 1.0 / 256.0, None,
                                    OP.mult)
            sgB = sc.tile([P, 1], F32, tag="sgB")
            nc.gpsimd.partition_broadcast(sgB[:], sg[:])
            bcast = sc.tile([P, T], F32, tag="bcast")
            nc.gpsimd.partition_broadcast(bcast[:], am_row[:])
            nc.vector.tensor_scalar(bcast[:], bcast[:], sgB[:], None, OP.mult)

            # ---------------- ternarize helper: f32 tile -> bf16 2*{-1,0,1}
            # 2*clip(rint(w*ws),-1,1) = Sign(w*ws-0.5) + Sign(w*ws+0.5)
            # (exact except measure-zero |w*ws|==0.5; the *2 is folded into
            #  the sg/sf dequant constants)
            _tz = [0]
            def ternarize(wt_f, dst_b, ws_ap):
                _tz[0] += 1
                ta = s2k.tile([P, 1024], BF16, tag="ta", name=f"ta{_tz[0]}")
                tb = s2k.tile([P, 1024], BF16, tag="tb", name=f"tb{_tz[0]}")
                nc.scalar.activation(ta[:], wt_f, ACT_FN.Sign,
                                     bias=halfn[:], scale=ws_ap)
                nc.scalar.activation(tb[:], wt_f, ACT_FN.Sign,
                                     bias=halfp[:], scale=ws_ap)
                nc.vector.tensor_tensor(dst_b, ta[:], tb[:], OP.add)

            # ---------------- GEMM1 + epilogue (H128 fused) -> interm bf16
            interm = ip.tile([P, C, T], BF16, tag="interm")
            wq = wqp.tile([P, 2, HC, P], BF16, tag="wq")
            for ic in range(C):
                for mi, w_v in ((0, wgt_v), (1, wut_v)):
                    for half in range(HH):
                        wt = wpipe.tile([P, 8, P], F32, tag="wf32",
                                        name=f"w{ic}_{mi}_{half}")
                        nc.sync.dma_start(
                            wt[:],
                            w_v[:, half * 8:(half + 1) * 8,
                                ic * P:(ic + 1) * P])
                        ternarize(
                            wt[:].rearrange("p a b -> p (a b)"),
                            wq[:, mi, half * 8:(half + 1) * 8, :].rearrange(
                                "p a b -> p (a b)"),
                            wsB[:, mi:mi + 1])
                ps_g = [ps_mm.tile([P, NT], F32, tag="psmm",
                                   name=f"psg{ic}_{t}") for t in range(TTN)]
                ps_u = [ps_mm.tile([P, NT], F32, tag="psmm",
                                   name=f"psu{ic}_{t}") for t in range(TTN)]
                for mi, ps in ((0, ps_g), (1, ps_u)):
                    for tt in range(TTN):
                        for hc in range(HC):
                            nc.tensor.matmul(
                                ps[tt][:], wq[:, mi, hc, :],
                                xqt[:, hc, tt * NT:(tt + 1) * NT],
                                start=(hc == 0), stop=(hc == HC - 1))
                for tt in range(TTN):
                    g1 = s2k.tile([P, NT], F32, tag="s2a", name=f"g1_{ic}{tt}")
                    nc.vector.tensor_tensor(g1[:], ps_g[tt][:],
                                            bcast[:, tt * NT:(tt + 1) * NT],
                                            OP.mult)
                    nc.scalar.activation(g1[:], g1[:], ACT_FN.Silu)
                    prod = s2k.tile([P, NT], F32, tag="s2b",
                                    name=f"pr_{ic}{tt}")
                    nc.vector.tensor_tensor(prod[:], g1[:], ps_u[tt][:],
                                            OP.mult)
                    psH = ps_h.tile([P, NT], F32, tag="psh")
                    nc.tensor.matmul(psH[:], hmat[:], prod[:],
                                     start=True, stop=True)
                    nc.scalar.copy(interm[:, ic, tt * NT:(tt + 1) * NT],
                                   psH[:])

            # ---------------- FWHT H64 butterflies + act-quant
            def fwht_quant(s_i, eng, ftag):
                cols = slice(s_i * sub, (s_i + 1) * sub)
                b0 = s2k.tile([P, C, sub], BF16, tag=f"{ftag}0",
                              name=f"fw0_{s_i}")
                b1 = s2k.tile([P, C, sub], BF16, tag=f"{ftag}1",
                              name=f"fw1_{s_i}")
                bufs = [b0, b1]
                src = interm[:, :, cols]
                for st in range(L):
                    h = 1 << st
                    dst = bufs[st % 2]
                    sview = src.rearrange("p (b two h) t -> p b two h t",
                                          two=2, h=h)
                    dview = dst[:].rearrange("p (b two h) t -> p b two h t",
                                             two=2, h=h)
                    eng.tensor_tensor(dview[:, :, 0], sview[:, :, 0],
                                      sview[:, :, 1], OP.add)
                    eng.tensor_tensor(dview[:, :, 1], sview[:, :, 0],
                                      sview[:, :, 1], OP.subtract)
                    src = dst[:]
                fin = bufs[(L - 1) % 2]          # butterfly output (bf16)
                oth = s2k.tile([P, C, sub], F32, tag=f"{ftag}{L % 2}",
                               name=f"oth{s_i}")  # f32 quant scratch
                m1r = sc.tile([P, sub], F32, tag="m1r", name=f"m1r{s_i}")
                nc.vector.tensor_reduce(
                    out=m1r[:], in_=fin[:].rearrange("p c t -> p t c"),
                    op=OP.max, axis=AX, apply_absolute_value=True)
                pmt = ps_tp.tile([P, P], F32, tag="tp", name=f"pm{s_i}")
                nc.tensor.transpose(pmt[:sub, :], m1r[:], ident_f[:])
                mrt = sc.tile([sub, P], F32, tag="mrt", name=f"mrt{s_i}")
                nc.scalar.copy(mrt[:], pmt[:sub, :])
                amc2 = sc.tile([sub, 1], F32, tag="amc2", name=f"am2{s_i}")
                nc.vector.tensor_reduce(out=amc2[:], in_=mrt[:], op=OP.max,
                                        axis=AX)
                nc.vector.tensor_scalar(amc2[:], amc2[:], EPS, None, OP.max)
                pr2 = ps_tp.tile([P, P], F32, tag="tp", name=f"pr2{s_i}")
                nc.tensor.transpose(pr2[:1, :sub], amc2[:],
                                    ident_f[:sub, :sub])
                # fold this slice's fwht absmax straight into the per-token
                # output scale row (am_row was pre-multiplied by sf)
                nc.vector.tensor_tensor(am_row[:, cols], am_row[:, cols],
                                        pr2[:1, :sub], OP.mult)
                s2row = sc.tile([1, sub], F32, tag="s2row", name=f"s2r{s_i}")
                nc.vector.reciprocal(s2row[:], pr2[:1, :sub])
                nc.vector.tensor_scalar(s2row[:], s2row[:], 128.0, None,
                                        OP.mult)
                ps2b = ps_tp.tile([P, P], F32, tag="tp", name=f"s2b{s_i}")
                nc.tensor.matmul(ps2b[:, :sub], ones1[:], s2row[:],
                                 start=True, stop=True)
                rc2 = sc.tile([P, sub], F32, tag="rc2", name=f"rc2{s_i}")
                nc.scalar.copy(rc2[:], ps2b[:, :sub])
                eng.tensor_tensor(
                    oth[:], fin[:],
                    rc2[:, None, :].to_broadcast((P, C, sub)), OP.mult)
                eng.tensor_scalar(oth[:], oth[:], QCLIP, MAGIC,
                                  OP.min, OP.add)
                nc.scalar.activation(interm[:, :, cols], oth[:],
                                     ACT_FN.Identity, bias=nmagicB[:])

            # final per-token output scale rows (per token-half so GEMM2 can
            # start after the first half)
            sf = sc.tile([1, 1], F32, tag="sf")
            nc.vector.tensor_tensor(sf[:], wm_row[:, 2:3], wm_row[:, 1:2],
                                    OP.mult)
            nc.vector.tensor_scalar(sf[:], sf[:], inv_sqrt_i / (128.0 * 128.0 * 4.0),
                                    None, OP.mult)
            nc.vector.tensor_scalar(am_row[:], am_row[:], sf[:], None, OP.mult)

            # ---------------- GEMM2 single-token-half pass (re-ternarizes
            # wd each pass so pass tt=0 can interleave with the second FWHT
            # half in every engine's program order)
            CG = 8                            # c-blocks per streamed wd group
            _g2 = [0]

            def g2_pass(hc, tt):
                _g2[0] += 1
                pso = ps_mm.tile([P, NT], F32, tag="psmm",
                                 name=f"pso{_g2[0]}")
                for g in range(C // CG):
                    wdq = wdqp.tile([P, CG, P], BF16, tag=f"wdq{g % 2}",
                                    name=f"wdq{_g2[0]}_{g}")
                    wt = wpipe.tile([P, 8, P], F32, tag="wf32",
                                    name=f"wd{_g2[0]}_{g}")
                    c0 = g * CG
                    nc.sync.dma_start(
                        wt[:],
                        wdt_v[:, c0:c0 + 8, hc * P:(hc + 1) * P])
                    ternarize(
                        wt[:].rearrange("p a b -> p (a b)"),
                        wdq[:].rearrange("p a b -> p (a b)"),
                        wsB[:, 2:3])
                    for cc in range(CG):
                        c = g * CG + cc
                        nc.tensor.matmul(
                            pso[:], wdq[:, cc, :],
                            interm[:, c, tt * NT:(tt + 1) * NT],
                            start=(c == 0), stop=(c == C - 1),
                            skip_group_check=True)
                o1 = s2k.tile([P, NT], F32, tag="o1", name=f"o{_g2[0]}")
                nc.vector.tensor_tensor(o1[:], pso[:],
                                        bcast[:, tt * NT:(tt + 1) * NT],
                                        OP.mult)
                for k in range(NT // P):
                    tb = tt * (NT // P) + k
                    po = ps_tp.tile([P, P], F32, tag="tp",
                                    name=f"po{_g2[0]}_{tb}")
                    nc.tensor.transpose(po[:], o1[:, k * P:(k + 1) * P],
                                        ident_f[:])
                    ot = otp.tile([P, P], F32, tag="ot",
                                  name=f"ot{_g2[0]}_{tb}")
                    nc.scalar.copy(ot[:], po[:])
                    nc.sync.dma_start(
                        out_d.ap()[tb * P:(tb + 1) * P,
                                   hc * P:(hc + 1) * P], ot[:])

            half_subs = n_sub // TTN
            # FWHT token-half 0
            for k in range(half_subs):
                fwht_quant(k, nc.vector, "s2")
            nc.gpsimd.partition_broadcast(bcast[:, 0:NT], am_row[:, 0:NT])
            # FWHT token-half 1 interleaved with GEMM2 over token-half 0
            per = half_subs // HC
            for hc in range(HC):
                for j in range(per):
                    fwht_quant(half_subs + hc * per + j, nc.vector, "s2")
                g2_pass(hc, 0)
            nc.gpsimd.partition_broadcast(bcast[:, NT:T], am_row[:, NT:T])
            # GEMM2 over token-half 1
            for hc in range(HC):
                g2_pass(hc, 1)

    nc.compile()
    return nc


_PROG_CACHE = {}
_LAST_IN_MAPS = None


def kernel(x, w_gate, w_up, w_down):
    from concourse.bass_utils import run_bass_kernel_spmd

    B, S, H = x.shape
    I = w_gate.shape[0]
    n_cores = 8
    M = B * S
    T = M // n_cores

    key = (T, H, I, n_cores)
    if key not in _PROG_CACHE:
        _PROG_CACHE[key] = build_program(T, H, I, n_cores)
    nc = _PROG_CACHE[key]

    xf = np.ascontiguousarray(x.reshape(M, H).astype(np.float32))
    wgT = np.ascontiguousarray(w_gate.T)     # [H, I]
    wuT = np.ascontiguousarray(w_up.T)       # [H, I]
    wdT = np.ascontiguousarray(w_down.T)     # [I, H]
    hm = hadamard128()
    HS, IS = H // n_cores, I // n_cores
    in_maps = []
    for c in range(n_cores):
        in_maps.append({
            "x_s": xf[c * T:(c + 1) * T],
            "wgt": wgT, "wut": wuT, "wdt": wdT,
            "wg_s": np.ascontiguousarray(wgT[c * HS:(c + 1) * HS]),
            "wu_s": np.ascontiguousarray(wuT[c * HS:(c + 1) * HS]),
            "wd_s": np.ascontiguousarray(wdT[c * IS:(c + 1) * IS]),
            "hmat": hm,
        })
    global _LAST_IN_MAPS
    _LAST_IN_MAPS = in_maps
    res = run_bass_kernel_spmd(nc, in_maps, list(range(n_cores)))
    out = np.concatenate([res.results[c]["out_s"] for c in range(n_cores)], 0)
    return out.reshape(B, S, H).astype(np.float32)



# revision 15
# speedup vs baseline: 1.3728x; 1.3728x over previous
"""BitNet MLP (ternary gate/up GEMM + silu*up + Hadamard + act-quant + down GEMM)
on 8 Trainium2 NeuronCores — v4.

Token-data-parallel (T=1024 tokens/core). v3 -> v4 (trace-driven):
  - butterflies are bit-exact via a spare-chunk ping-pong (2 tensor_tensor
    ops per pair, physical-slot indirection) instead of the lossy 2a-(a-b)
    form; still run in-flight during GEMM1, burst-smoothed,
  - per-token absmax of the intermediate is accumulated CONTIGUOUSLY
    (abs_max accumulate after each stage-6 pair) instead of one giant
    strided reduce,
  - intermediate act-quant is exact f32 (PSUM-staged MAGIC rint, matching
    v2 numerics) emitted chunk-major so GEMM2's hc=0 pipelines behind the
    quant wavefront,
  - GEMM1 emits all four matmul groups before the epilogues and defers the
    tt1 H128 matmul into the next chunk's PE stream (no FIFO stall on the
    epilogue chain); H128 runs on float32r (1 cyc/row),
  - GEMM2 is hc-outer with both token halves interleaved per wd piece so
    w_down is DMA'd + ternarized exactly once; ternarize add on DVE.
"""

import sys

sys.path.insert(0, "/opt/trn_rl_repo")

import numpy as np

import concourse.bass as bass
import concourse.mybir as mybir
import concourse.tile as tile
from concourse import bacc, bass_isa

F32 = mybir.dt.float32
F32R = mybir.dt.float32r
BF16 = mybir.dt.bfloat16
AX = mybir.AxisListType.X
OP = mybir.AluOpType
ACT_FN = mybir.ActivationFunctionType

MAGIC = 12582912.0  # 1.5*2^23: (x + MAGIC) - MAGIC == rint(x) in f32
EPS = 1e-5
QCLIP = 127.4375


def hadamard128():
    h = np.array([[1.0]], dtype=np.float32)
    while h.shape[0] < 128:
        h = np.block([[h, h], [h, -h]]).astype(np.float32)
    return h


def build_program(T, H, I, n_cores):
    from concourse.masks import make_identity

    P = 128
    HC = H // P                    # 16 h-chunks
    C = I // P                     # 64 i-chunks (fwht H64 factor)
    NT = min(512, T)
    TTN = T // NT                  # 2 token halves
    n_tb = T // P                  # 8 token 128-blocks
    L = int(np.log2(C))            # 6 butterfly stages over the chunk index
    assert 2 ** L == C and T % P == 0 and H % 1024 == 0 and I % P == 0
    inv_sqrt_i = float(1.0 / np.sqrt(I))
    wcount = float(I) * float(H)

    nc = bacc.Bacc("TRN2", target_bir_lowering=False, num_devices=n_cores)

    x_d = nc.dram_tensor("x_s", [T, H], F32, kind="ExternalInput")
    wgt_d = nc.dram_tensor("wgt", [H, I], F32, kind="ExternalInput")
    wut_d = nc.dram_tensor("wut", [H, I], F32, kind="ExternalInput")
    wdt_d = nc.dram_tensor("wdt", [I, H], F32, kind="ExternalInput")
    wgs_d = nc.dram_tensor("wg_s", [H // n_cores, I], F32, kind="ExternalInput")
    wus_d = nc.dram_tensor("wu_s", [H // n_cores, I], F32, kind="ExternalInput")
    wds_d = nc.dram_tensor("wd_s", [I // n_cores, H], F32, kind="ExternalInput")
    hm_d = nc.dram_tensor("hmat", [P, P], F32, kind="ExternalInput")
    out_d = nc.dram_tensor("out_s", [T, H], F32, kind="ExternalOutput")

    cc_in = nc.dram_tensor("cc_in", [1, 4], F32)
    cc_out = nc.dram_tensor("cc_out", [1, 4], F32, addr_space="Shared")

    # DRAM views for transposed-tile loads: [p, q, i] with h = q*128 + p
    wgt_v = wgt_d.ap().rearrange("(q p) i -> p q i", p=P)
    wut_v = wut_d.ap().rearrange("(q p) i -> p q i", p=P)
    wdt_v = wdt_d.ap().rearrange("(c p) h -> p c h", p=P)

    with tile.TileContext(nc) as tc:
        with (
            tc.tile_pool(name="consts", bufs=1) as consts,
            tc.tile_pool(name="wpipe", bufs=2) as wpipe,    # f32 stage [P,8,128]
            tc.tile_pool(name="wqp", bufs=1) as wqp,        # ternarized weights
            tc.tile_pool(name="xq", bufs=1) as xqp,         # xqT persistent
            tc.tile_pool(name="ip", bufs=1) as ip,          # interm
            tc.tile_pool(name="s2k", bufs=1) as s2k,        # scratch
            tc.tile_pool(name="sc", bufs=1) as sc,          # scales/rows
            tc.tile_pool(name="ps", bufs=1, space="PSUM") as psp,
        ):
            # ---------------- constants
            hmtmp = wpipe.tile([P, 8, P], F32, tag="wf32", name="hmtmp")
            nc.sync.dma_start(hmtmp[:, 0, :], hm_d.ap())
            ident_f = consts.tile([P, P], F32, tag="ident_f")
            make_identity(nc, ident_f[:])
            ident_b = consts.tile([P, P], BF16, tag="ident_b")
            nc.vector.tensor_copy(ident_b[:], ident_f[:])
            hmat_r = consts.tile([P, P], F32R, tag="hmat_r")
            nc.vector.tensor_copy(hmat_r[:], hmtmp[:, 0, :])
            magicB = consts.tile([P, 1], F32, tag="magicB")
            nc.vector.memset(magicB[:], MAGIC)
            nmagicB = consts.tile([P, 1], F32, tag="nmagicB")
            nc.vector.memset(nmagicB[:], -MAGIC)
            ones1 = consts.tile([1, P], F32, tag="ones1")
            nc.vector.memset(ones1[:], 1.0)
            halfn = consts.tile([P, 1], F32, tag="halfn")
            nc.vector.memset(halfn[:], -0.5)
            halfp = consts.tile([P, 1], F32, tag="halfp")
            nc.vector.memset(halfp[:], 0.5)
            m1 = sc.tile([P, T], BF16, tag="m1")
            nc.vector.memset(m1[:], 0.0)

            # ---------------- weight-scale pass (shard |w| sums + AllReduce)
            def shard_abs_sum(src_d, rows, cols, tag):
                ntr, ntc = rows // P, cols // 1024
                acc = sc.tile([P, ntr * ntc], F32, tag=f"acc_{tag}")
                for r in range(ntr):
                    for q in range(ntc):
                        t = wpipe.tile([P, 8, P], F32, tag="wf32",
                                       name=f"as_{tag}_{r}_{q}")
                        nc.sync.dma_start(
                            t[:].rearrange("p a b -> p (a b)"),
                            src_d.ap()[r * P:(r + 1) * P,
                                       q * 1024:(q + 1) * 1024])
                        nc.vector.tensor_reduce(
                            out=acc[:, r * ntc + q:r * ntc + q + 1],
                            in_=t[:].rearrange("p a b -> p (a b)"),
                            op=OP.add, axis=AX, apply_absolute_value=True)
                tot = sc.tile([P, 1], F32, tag=f"tot_{tag}")
                nc.vector.tensor_reduce(out=tot[:], in_=acc[:], op=OP.add,
                                        axis=AX)
                red = sc.tile([P, 1], F32, tag=f"red_{tag}")
                nc.gpsimd.partition_all_reduce(
                    red[:], tot[:], channels=P, reduce_op=bass_isa.ReduceOp.add)
                return red

            red_g = shard_abs_sum(wgs_d, H // n_cores, I, "g")
            red_u = shard_abs_sum(wus_d, H // n_cores, I, "u")
            red_d = shard_abs_sum(wds_d, I // n_cores, H, "d")

            ccin_sb = sc.tile([1, 4], F32, tag="ccin")
            nc.vector.memset(ccin_sb[:], 0.0)
            nc.vector.tensor_copy(ccin_sb[:, 0:1], red_g[0:1, :])
            nc.vector.tensor_copy(ccin_sb[:, 1:2], red_u[0:1, :])
            nc.vector.tensor_copy(ccin_sb[:, 2:3], red_d[0:1, :])
            nc.sync.dma_start(cc_in.ap(), ccin_sb[:])
            nc.gpsimd.collective_compute(
                "AllReduce", OP.add, ins=[cc_in.ap()], outs=[cc_out.ap()],
                replica_groups=[list(range(n_cores))])
            sums_sb = sc.tile([1, 4], F32, tag="sums")
            nc.sync.dma_start(sums_sb[:], cc_out.ap())

            # ---------------- x: act-quant + transpose into xqT (bf16 ints)
            xqt = xqp.tile([P, HC, T], BF16, tag="xqt")
            am_row = sc.tile([1, T], F32, tag="am_row")
            for tb in range(n_tb):
                xt = [wpipe.tile([P, 8, P], F32, tag="wf32", name=f"x{tb}_{q}")
                      for q in range(2)]
                for q in range(2):
                    nc.sync.dma_start(
                        xt[q][:].rearrange("p a b -> p (a b)"),
                        x_d.ap()[tb * P:(tb + 1) * P,
                                 q * 1024:(q + 1) * 1024])
                am2 = sc.tile([P, 2], F32, tag="am2")
                for q in range(2):
                    nc.vector.tensor_reduce(
                        out=am2[:, q:q + 1],
                        in_=xt[q][:].rearrange("p a b -> p (a b)"),
                        op=OP.max, axis=AX, apply_absolute_value=True)
                amc = sc.tile([P, 1], F32, tag="amc")
                nc.vector.tensor_reduce(out=amc[:], in_=am2[:], op=OP.max,
                                        axis=AX)
                nc.vector.tensor_scalar(amc[:], amc[:], EPS, None, OP.max)
                sx = sc.tile([P, 1], F32, tag="sx")
                nc.vector.reciprocal(sx[:], amc[:])
                nc.vector.tensor_scalar(sx[:], sx[:], 128.0, None, OP.mult)
                for q in range(2):
                    f = xt[q][:].rearrange("p a b -> p (a b)")
                    nc.scalar.activation(f, f, ACT_FN.Identity,
                                         bias=magicB[:], scale=sx[:])
                    xqb = s2k.tile([P, 1024], BF16, tag="ta",
                                   name=f"xqb{tb}_{q}")
                    nc.vector.tensor_scalar(xqb[:], f, -MAGIC, 127.0,
                                            OP.add, OP.min)
                    for k in range(8):
                        hc = q * 8 + k
                        pt = psp.tile([P, P], BF16, tag="tp", bufs=2,
                                      name=f"pt{tb}_{hc}")
                        nc.tensor.transpose(pt[:], xqb[:, k * P:(k + 1) * P],
                                            ident_b[:])
                        nc.scalar.copy(xqt[:, hc, tb * P:(tb + 1) * P], pt[:])
                pr = psp.tile([P, P], F32, tag="tp", bufs=2, name=f"pr{tb}")
                nc.tensor.transpose(pr[:1, :], amc[:], ident_f[:])
                nc.scalar.copy(am_row[:, tb * P:(tb + 1) * P], pr[:1, :])

            # ---------------- finalize weight scales (waits on AllReduce)
            wm_row = sc.tile([1, 4], F32, tag="wm_row")
            nc.vector.tensor_scalar(wm_row[:], sums_sb[:], 1.0 / wcount, EPS,
                                    OP.mult, OP.max)
            ws_row = sc.tile([1, 4], F32, tag="ws_row")
            nc.vector.reciprocal(ws_row[:], wm_row[:])
            wsB = sc.tile([P, 4], F32, tag="wsB")
            nc.gpsimd.partition_broadcast(wsB[:], ws_row[:])

            # bcast = per-token gate dequant (am * wm_g / 256), all partitions
            sg = sc.tile([1, 1], F32, tag="sg")
            nc.vector.tensor_scalar(sg[:], wm_row[:, 0:1], 1.0 / 256.0, None,
                                    OP.mult)
            sgB = sc.tile([P, 1], F32, tag="sgB")
            nc.gpsimd.partition_broadcast(sgB[:], sg[:])
            bcast = sc.tile([P, T], F32, tag="bcast")
            nc.gpsimd.partition_broadcast(bcast[:], am_row[:])
            nc.vector.tensor_scalar(bcast[:], bcast[:], sgB[:], None, OP.mult)

            # ---------------- ternarize helper: f32 tile -> bf16 2*{-1,0,1}
            # 2*clip(rint(w*ws),-1,1) = Sign(w*ws-0.5) + Sign(w*ws+0.5)
            _tz = [0]
            def ternarize(wt_f, dst_b, ws_ap, add_eng):
                _tz[0] += 1
                ta = s2k.tile([P, 1024], BF16, tag="ta", name=f"ta{_tz[0]}")
                nc.scalar.activation(dst_b, wt_f, ACT_FN.Sign,
                                     bias=halfn[:], scale=ws_ap)
                nc.scalar.activation(ta[:], wt_f, ACT_FN.Sign,
                                     bias=halfp[:], scale=ws_ap)
                add_eng.tensor_tensor(dst_b, dst_b, ta[:], OP.add)

            # ---------------- in-flight H64 butterfly, bit-exact via a
            # spare physical chunk slot: s = a+b -> spare ; b = a-b in place;
            # the old a-slot becomes the new spare.
            interm = ip.tile([P, C + 1, T], BF16, tag="interm")
            loc = list(range(C))
            spare = [C]

            def butterfly(stage, a_idx, b_idx):
                pa, pb, sp = loc[a_idx], loc[b_idx], spare[0]
                A = interm[:, pa, :]
                B = interm[:, pb, :]
                S_ = interm[:, sp, :]
                nc.vector.tensor_tensor(S_, A, B, OP.add)
                nc.vector.tensor_tensor(B, A, B, OP.subtract)
                loc[a_idx] = sp
                spare[0] = pa
                if stage == L:
                    # last stage: accumulate per-(partition,token) max into
                    # m1 and min into bcast (re-used as scratch; its gate
                    # scales are dead after the last GEMM1 epilogue)
                    for pidx in (loc[a_idx], loc[b_idx]):
                        nc.vector.tensor_tensor(m1[:], m1[:],
                                                interm[:, pidx, :], OP.max)
                        nc.vector.tensor_tensor(bcast[:], bcast[:],
                                                interm[:, pidx, :], OP.min)

            pending = []  # butterfly pairs not yet emitted (burst smoothing)

            # ---------------- GEMM1 + epilogue (H128 fused) -> interm bf16
            def epi_dve(ic, tt, ps_g, ps_u):
                ts = slice(tt * NT, (tt + 1) * NT)
                g1 = s2k.tile([P, NT], F32R, tag="g1", bufs=2,
                              name=f"g1_{ic}{tt}")
                nc.vector.tensor_tensor(g1[:], ps_g[:], bcast[:, ts],
                                        OP.mult)
                nc.scalar.activation(g1[:], g1[:], ACT_FN.Silu)
                nc.vector.tensor_tensor(g1[:], g1[:], ps_u[:], OP.mult)
                return g1

            def epi_h128(ic, tt, g1):
                ts = slice(tt * NT, (tt + 1) * NT)
                psH = psp.tile([P, NT], F32, tag="mm", bufs=6,
                               name=f"psh{ic}_{tt}")
                nc.tensor.matmul(psH[:], hmat_r[:], g1[:],
                                 start=True, stop=True)
                nc.scalar.copy(interm[:, ic, ts], psH[:])

            pend_h = [None]
            for ic in range(C):
                wq = []
                for mi, w_v in ((0, wgt_v), (1, wut_v)):
                    w = wqp.tile([P, HC, P], BF16, tag=f"wq{mi}",
                                 bufs=(2 if mi == 0 else 1),
                                 name=f"wq{mi}_{ic}")
                    wq.append(w)
                    for half in range(2):
                        wt = wpipe.tile([P, 8, P], F32, tag="wf32",
                                        name=f"w{ic}_{mi}_{half}")
                        nc.sync.dma_start(
                            wt[:],
                            w_v[:, half * 8:(half + 1) * 8,
                                ic * P:(ic + 1) * P])
                        ternarize(
                            wt[:].rearrange("p a b -> p (a b)"),
                            w[:, half * 8:(half + 1) * 8, :].rearrange(
                                "p a b -> p (a b)"),
                            wsB[:, mi:mi + 1], nc.vector)
                ps = []
                for tt in range(TTN):
                    ts = slice(tt * NT, (tt + 1) * NT)
                    ps_g = psp.tile([P, NT], F32, tag="mm", bufs=6,
                                    name=f"psg{ic}_{tt}")
                    ps_u = psp.tile([P, NT], F32, tag="mm", bufs=6,
                                    name=f"psu{ic}_{tt}")
                    ps.append((ps_g, ps_u))
                    for hc in range(HC):
                        nc.tensor.matmul(ps_g[:], wq[0][:, hc, :],
                                         xqt[:, hc, ts],
                                         start=(hc == 0), stop=(hc == HC - 1))
                    for hc in range(HC):
                        nc.tensor.matmul(ps_u[:], wq[1][:, hc, :],
                                         xqt[:, hc, ts],
                                         start=(hc == 0), stop=(hc == HC - 1))
                    if tt == 0:
                        # H128 of the previous chunk's tt1 (deferred: its
                        # product is ready by now; 2 mm groups of slack)
                        if pend_h[0] is not None:
                            epi_h128(*pend_h[0])
                            pend_h[0] = None
                        g1_0 = epi_dve(ic, 0, ps_g, ps_u)
                    else:
                        epi_h128(ic, 0, g1_0)
                        g1_1 = epi_dve(ic, 1, ps_g, ps_u)
                        pend_h[0] = (ic, 1, g1_1)
                    # smoothed butterfly emission
                    for _ in range(2):
                        if pending:
                            butterfly(*pending.pop(0))
                # queue butterfly stages whose window closes at this chunk
                for s in range(1, L + 1):
                    span = 1 << s
                    if (ic + 1) % span == 0:
                        base = ic + 1 - span
                        hs = span // 2
                        for k in range(hs):
                            pending.append((s, base + k, base + k + hs))
            if pend_h[0] is not None:
                epi_h128(*pend_h[0])
                pend_h[0] = None

            # GEMM2 wd prefetch for the first pieces (DMA + ternarize run
            # during the quant tail; only 2 pieces fit the wdq rotation)
            def g2_piece(hc, piece, add_eng):
                wdq = wqp.tile([P, 8, P], BF16, tag="wq0", bufs=2,
                               name=f"wdq{hc}_{piece}")
                wt = wpipe.tile([P, 8, P], F32, tag="wf32",
                                name=f"wd{hc}_{piece}")
                c0 = piece * 8
                nc.sync.dma_start(
                    wt[:], wdt_v[:, c0:c0 + 8, hc * P:(hc + 1) * P])
                ternarize(wt[:].rearrange("p a b -> p (a b)"),
                          wdq[:].rearrange("p a b -> p (a b)"),
                          wsB[:, 2:3], add_eng)
                return wdq

            # flush remaining butterflies (the last-block cascade + stage 6
            # with fused absmax accumulation)
            while pending:
                butterfly(*pending.pop(0))

            wdq_pre = [g2_piece(0, 0, nc.gpsimd), g2_piece(0, 1, nc.gpsimd)]

            # ---------------- per-token scales for the intermediate quant
            sf = sc.tile([1, 1], F32, tag="sf")
            nc.vector.tensor_tensor(sf[:], wm_row[:, 2:3], wm_row[:, 1:2],
                                    OP.mult)
            nc.vector.tensor_scalar(sf[:], sf[:],
                                    inv_sqrt_i / (128.0 * 128.0 * 4.0),
                                    None, OP.mult)
            nc.vector.tensor_scalar(am_row[:], am_row[:], sf[:], None, OP.mult)

            # merged per-(partition,token) absmax: bcast = max(-min, max)
            nc.vector.scalar_tensor_tensor(bcast[:], bcast[:], -1.0, m1[:],
                                           OP.mult, OP.max)

            rcs = []
            for tt in range(TTN):
                ts = slice(tt * NT, (tt + 1) * NT)
                nb = NT // P
                amT = sc.tile([P, nb], F32, tag="amT", name=f"amT_{tt}")
                for k in range(nb):
                    ptf = psp.tile([P, P], F32, tag="tp", bufs=2,
                                   name=f"qpt{tt}_{k}")
                    nc.tensor.transpose(
                        ptf[:],
                        bcast[:, tt * NT + k * P:tt * NT + (k + 1) * P],
                        ident_f[:])
                    nc.vector.tensor_reduce(out=amT[:, k:k + 1], in_=ptf[:],
                                            op=OP.max, axis=AX)
                nc.vector.tensor_scalar(amT[:], amT[:], EPS, None, OP.max)
                sT = sc.tile([P, nb], F32, tag="sT", name=f"sT_{tt}")
                nc.vector.reciprocal(sT[:], amT[:])
                nc.vector.tensor_scalar(sT[:], sT[:], 128.0, None, OP.mult)
                srow = sc.tile([1, NT], F32, tag="srow", name=f"srow_{tt}")
                for k in range(nb):
                    cols = slice(tt * NT + k * P, tt * NT + (k + 1) * P)
                    prk = psp.tile([P, P], F32, tag="tp", bufs=2,
                                   name=f"prk{tt}_{k}")
                    nc.tensor.transpose(prk[:1, :], amT[:, k:k + 1],
                                        ident_f[:])
                    nc.vector.tensor_tensor(am_row[:, cols], am_row[:, cols],
                                            prk[:1, :], OP.mult)
                    psk = psp.tile([P, P], F32, tag="tp", bufs=2,
                                   name=f"psk{tt}_{k}")
                    nc.tensor.transpose(psk[:1, :], sT[:, k:k + 1],
                                        ident_f[:])
                    nc.scalar.copy(srow[:, k * P:(k + 1) * P], psk[:1, :])
                psb = psp.tile([P, NT], F32, tag="mm", bufs=6,
                               name=f"psb_{tt}")
                nc.tensor.matmul(psb[:], ones1[:], srow[:],
                                 start=True, stop=True)
                rc = s2k.tile([P, NT], F32, tag="g1", bufs=2,
                              name=f"rc_{tt}")
                nc.scalar.copy(rc[:], psb[:])
                rcs.append(rc)
                # refresh the per-token output-dequant broadcast for GEMM2
                nc.gpsimd.partition_broadcast(bcast[:, ts], am_row[:, ts])

            # ---------------- intermediate act-quant: exact f32 rint via
            # PSUM staging, chunk-major so GEMM2 pipelines right behind it.
            # hc=0's remaining wd pieces are emitted inside the wavefront so
            # their ternarize tracks the quant progress.
            for c in range(C):
                pc = loc[c]
                for tt in range(TTN):
                    ts = slice(tt * NT, (tt + 1) * NT)
                    psq = psp.tile([P, NT], F32, tag="mm", bufs=6,
                                   name=f"q{c}_{tt}")
                    nc.vector.tensor_tensor(psq[:], interm[:, pc, ts],
                                              rcs[tt][:], OP.mult)
                    nc.vector.tensor_scalar(psq[:], psq[:], QCLIP, MAGIC,
                                            OP.min, OP.add)
                    nc.scalar.activation(interm[:, pc, ts], psq[:],
                                         ACT_FN.Identity, bias=nmagicB[:])
                if (c + 1) % 8 == 0 and 2 <= (c + 1) // 8 <= 7:
                    wdq_pre.append(g2_piece(0, (c + 1) // 8, nc.gpsimd))

            # ---------------- GEMM2: hc-outer, both token halves interleaved
            # (wd DMA'd + ternarized exactly once)
            for hc in range(HC):
                pso = [psp.tile([P, NT], F32, tag="mm", bufs=6,
                                name=f"pso{hc}_{t}") for t in range(TTN)]
                for piece in range(8):
                    if hc == 0:
                        wdq = wdq_pre[piece]
                    else:
                        wdq = g2_piece(hc, piece, nc.vector)
                    for j in range(8):
                        c = piece * 8 + j
                        for tt in range(TTN):
                            nc.tensor.matmul(
                                pso[tt][:], wdq[:, j, :],
                                interm[:, loc[c], tt * NT:(tt + 1) * NT],
                                start=(c == 0), stop=(c == C - 1),
                                skip_group_check=True)
                for tt in range(TTN):
                    o1 = s2k.tile([P, NT], F32, tag="g1", bufs=2,
                                  name=f"o{hc}_{tt}")
                    nc.vector.tensor_tensor(o1[:], pso[tt][:],
                                            bcast[:, tt * NT:(tt + 1) * NT],
                                            OP.mult)
                    for k in range(NT // P):
                        tb = tt * (NT // P) + k
                        po = psp.tile([P, P], F32, tag="tp", bufs=2,
                                      name=f"po{hc}_{tb}")
                        nc.tensor.transpose(po[:], o1[:, k * P:(k + 1) * P],
                                            ident_f[:])
                        ot = s2k.tile([P, P], F32, tag="ot", bufs=2,
                                      name=f"ot{hc}_{tb}")
                        nc.scalar.copy(ot[:], po[:])
                        nc.sync.dma_start(
                            out_d.ap()[tb * P:(tb + 1) * P,
                                       hc * P:(hc + 1) * P], ot[:])

    nc.compile()
    return nc


_PROG_CACHE = {}
_LAST_IN_MAPS = None


def kernel(x, w_gate, w_up, w_down):
    from concourse.bass_utils import run_bass_kernel_spmd

    B, S, H = x.shape
    I = w_gate.shape[0]
    n_cores = 8
    M = B * S
    T = M // n_cores

    key = (T, H, I, n_cores)
    if key not in _PROG_CACHE:
        _PROG_CACHE[key] = build_program(T, H, I, n_cores)
    nc = _PROG_CACHE[key]

    xf = np.ascontiguousarray(x.reshape(M, H).astype(np.float32))
    wgT = np.ascontiguousarray(w_gate.T)     # [H, I]
    wuT = np.ascontiguousarray(w_up.T)       # [H, I]
    wdT = np.ascontiguousarray(w_down.T)     # [I, H]
    hm = hadamard128()
    HS, IS = H // n_cores, I // n_cores
    in_maps = []
    for c in range(n_cores):
        in_maps.append({
            "x_s": xf[c * T:(c + 1) * T],
            "wgt": wgT, "wut": wuT, "wdt": wdT,
            "wg_s": np.ascontiguousarray(wgT[c * HS:(c + 1) * HS]),
            "wu_s": np.ascontiguousarray(wuT[c * HS:(c + 1) * HS]),
            "wd_s": np.ascontiguousarray(wdT[c * IS:(c + 1) * IS]),
            "hmat": hm,
        })
    global _LAST_IN_MAPS
    _LAST_IN_MAPS = in_maps
    res = run_bass_kernel_spmd(nc, in_maps, list(range(n_cores)))
    out = np.concatenate([res.results[c]["out_s"] for c in range(n_cores)], 0)
    return out.reshape(B, S, H).astype(np.float32)
